# revision 23
# baseline (speedup 1.0000x reference)
"""GCN autoencoder forward pass on 8 Trainium2 NeuronCores (Bass/Tile).

Strategy (graph/data parallel per the sharding hint):
  - Nodes permuted by in-degree, dealt to 8 cores tile-round-robin
    (128-node tiles -> near-uniform per-tile degree, balanced edges).
  - Per conv layer: per-core matmul produces table rows m = dinv*(act @ W)
    in fp16; the shard tables are AllGathered (halo exchange) into a
    Shared-scratchpad full table in FOUR window chunks (windows = 25/25/
    24/24 tile ranges) so each chunk overlaps the producing phase and the
    consuming gathers.
  - Aggregation is gather + matmul: per block of 8 destination tiles, the
    block's in-edge messages are staged into SBUF with a handful of
    SWDGE dma_gather instructions (dense 128-slot columns; indices are
    window-local int16), a 0/1 selection matrix S[slot, dst] is built on
    the DVE with one is_equal per chunk, and TensorE matmuls
    acc[dst,:] += S_c^T @ stage_c accumulate each tile's messages in
    PSUM (four tiles share one 2KB bank as slices of one accumulation
    group - only the globally-first matmul uses start=True).  This
    replaces per-edge-wave indirect DMAs (~1us SWDGE fixed cost each)
    and the DVE fold tree of the previous revision.
  - t2 (64-wide) rows are zero-padded to 128 so all tables gather with
    elem_size=128; t3 holds q = dinv*(LN(z) @ W1d) (the 5->128 matmul is
    hoisted BEFORE aggregation - matmul and scatter-add commute), so the
    decoder aggregates 128-wide as well.
  - The 5-wide bottleneck (relu->linear->layernorm) is computed batched
    per window chunk (amortizes DVE/Scalar instruction overheads).
  - Tables, bounces, weights and matmul operands are fp16 (rel err
    ~1e-3, tolerance 2e-2); aggregation accumulates fp32 in PSUM.

Self-contained: includes the walrus sync-wait compat shim and a PJRT
runner (axon) replicating bass2jax.run_bass_via_pjrt.
"""

import sys

for _p in ("/opt/trn_rl_repo",):
    if _p not in sys.path:
        sys.path.insert(0, _p)

import numpy as np

import concourse.bass as bass
import concourse.mybir as mybir
import concourse.tile as tile
from concourse import library_config
from concourse.masks import make_identity

P = 128
NCORES = 8
N = 100000
TILES_PER_CORE = 98
SHARD = TILES_PER_CORE * P          # 12544
NPAD = NCORES * SHARD               # 100352
F1, F2, FZ, FO = 128, 64, 5, 128
EPS = 1e-5
AF = mybir.ActivationFunctionType
F16 = mybir.dt.float16
F32 = mybir.dt.float32

BLK = 8                              # dst tiles per aggregation block
# AllGather window sizes (tiles).  Uneven on purpose: big early windows can
# fire their chunk ~1/3 into the previous phase; the last window - whose
# chunk is unavoidably serialized between layers - is small.  Each window
# must stay under 32 tiles (32768 rows) for int16 gather indices.
WT = (31, 31, 22, 14)
WOFF = (0, 31, 62, 84, 98)
WROWS = tuple(NCORES * t * P for t in WT)      # rows per window
WBASE = (0, 31744, 63488, 86016)               # table row base per window
HOOK_BLOCKS = tuple((WOFF[s + 1] - 1) // BLK for s in range(3))  # (3, 7, 10)
CHUNK_COLS = 80                      # stage/S chunk size (columns)
PIECE_COLS = 8                       # columns per dma_gather (1024 descs =
                                     # the hard SWDGE ring capacity)
NQ = 4                               # SWDGE queues (gathers round-robin)
PAD_DST = 255                        # dst id for padding slots

# ---------------------------------------------------------------- compat ----

MAX_WAITS = 1


def _split_sync_waits(nc, max_waits=MAX_WAITS):
    """This container's walrus rejects >1 sync wait per instruction; move
    excess waits onto same-engine NOPs placed just before the offender."""
    for fn in nc.m.functions:
        for bb in fn.blocks:
            out = []
            for inst in bb.instructions:
                si = inst.sync_info
                if si is not None and si.on_wait and len(si.on_wait) > max_waits:
                    waits = list(si.on_wait)
                    head, tail = waits[:-max_waits], waits[-max_waits:]
                    for i in range(0, len(head), max_waits):
                        out.append(
                            mybir.InstNoOp(
                                name=f"{inst.name}-ws{i}",
                                engine=inst.engine,
                                bass_nofuse=True,
                                sync_info=mybir.SyncInfo(
                                    on_wait=head[i : i + max_waits], on_update=[]
                                ),
                            )
                        )
                    si.on_wait = tail
                out.append(inst)
            bb.instructions[:] = out


class CompatTileContext(tile.TileContext):
    def __exit__(self, *args):
        ret = super().__exit__(*args)
        _split_sync_waits(self.nc)
        return ret


# ---------------------------------------------------------------- runner ----


class SpmdRunner:
    def __init__(self, nc, n_cores=NCORES):
        import jax
        from jax.sharding import Mesh, PartitionSpec, NamedSharding
        from jax.experimental.shard_map import shard_map
        from concourse import bass2jax
        from concourse.bass2jax import _bass_exec_p, install_neuronx_cc_hook

        install_neuronx_cc_hook()
        mybir.codegen_inst_isa_subclasses(nc)
        self.jax = jax
        self.nc = nc
        self.n_cores = n_cores
        partition_name = (
            nc.partition_id_tensor.name if nc.partition_id_tensor else None
        )

        in_names, out_names, out_avals, zero_outs = [], [], [], []
        for alloc in nc.m.functions[0].allocations:
            if not isinstance(alloc, mybir.MemoryLocationSet):
                continue
            name = alloc.memorylocations[0].name
            if alloc.kind == "ExternalInput":
                if name != partition_name:
                    in_names.append(name)
            elif alloc.kind == "ExternalOutput":
                out_names.append(name)
                shape = tuple(alloc.tensor_shape)
                dtype = mybir.dt.np(alloc.dtype)
                out_avals.append(jax.core.ShapedArray(shape, dtype))
                zero_outs.append(np.zeros(shape, dtype))
        self.in_names = in_names
        self.out_names = out_names
        self.out_avals = out_avals
        self.zero_outs = zero_outs
        n_params = len(in_names)
        all_in_names = in_names + out_names
        if partition_name is not None:
            all_in_names = all_in_names + [partition_name]

        def _body(*args):
            operands = list(args)
            if partition_name is not None:
                operands.append(bass2jax.partition_id_tensor())
            outs = _bass_exec_p.bind(
                *operands,
                out_avals=tuple(out_avals),
                in_names=tuple(all_in_names),
                out_names=tuple(out_names),
                lowering_input_output_aliases=(),
                sim_require_finite=True,
                sim_require_nnan=True,
                nc=nc,
            )
            return tuple(outs)

        devices = jax.devices()[:n_cores]
        self.mesh = Mesh(np.asarray(devices), ("core",))
        in_specs = (PartitionSpec("core"),) * (n_params + len(out_names))
        out_specs = (PartitionSpec("core"),) * len(out_names)
        self.sharding = NamedSharding(self.mesh, PartitionSpec("core"))
        self.fn = jax.jit(
            shard_map(_body, mesh=self.mesh, in_specs=in_specs,
                      out_specs=out_specs, check_rep=False),
            keep_unused=True,
        )
        self._dev_args = None

    def stage(self, in_maps):
        self._staged_in_maps = in_maps
        n = self.n_cores
        concat = [
            np.concatenate([np.asarray(in_maps[c][name]) for c in range(n)], axis=0)
            for name in self.in_names
        ]
        concat += [
            np.zeros((n * z.shape[0], *z.shape[1:]), z.dtype) for z in self.zero_outs
        ]
        self._dev_args = [self.jax.device_put(a, self.sharding) for a in concat]
        for a in self._dev_args:
            a.block_until_ready()

    def run(self):
        outs = self.fn(*self._dev_args)
        self.jax.block_until_ready(outs)
        return outs

    def results(self, outs):
        res = []
        for c in range(self.n_cores):
            d = {}
            for i, name in enumerate(self.out_names):
                full = np.asarray(outs[i])
                d[name] = full.reshape(self.n_cores, *self.out_avals[i].shape)[c]
            res.append(d)
        return res


# ------------------------------------------------------------------ plan ----

WINDOW_OF_TILE = np.concatenate(
    [np.full(WT[s], s, dtype=np.int64) for s in range(4)])


def layout_from_colspec(colspec):
    """colspec: per block, per window, list of (tile, ncols).  Returns the
    global column layout shared by the host packer and the program builder:
      blocks: list of dicts with
        tiles: list of tile ids in the block
        pieces: [(s, c0, c1)] gather instructions (global col ranges)
        chunks: [(c0, c1, [(s, pc0, pc1), ...])] stage/S granularity
      owner[c]: tile owning global column c
      tile_ncols[i]: total columns of tile i
      total_cols
    """
    owner = []
    blocks = []
    tile_ncols = {}
    c = 0
    for bi, bw in enumerate(colspec):
        b0 = bi * BLK
        tiles = list(range(b0, min(b0 + BLK, TILES_PER_CORE)))
        pieces = []
        for s in range(4):
            g0 = c
            for (i, ncols) in bw[s]:
                owner.extend([i] * ncols)
                tile_ncols[i] = tile_ncols.get(i, 0) + ncols
                c += ncols
            # split the window group into <= PIECE_COLS gather pieces
            p0 = g0
            while p0 < c:
                p1 = min(p0 + PIECE_COLS, c)
                pieces.append((s, p0, p1))
                p0 = p1
        # greedy merge consecutive pieces into chunks of <= CHUNK_COLS
        chunks = []
        cur = []
        cur_n = 0
        for (s, p0, p1) in pieces:
            n = p1 - p0
            if cur and cur_n + n > CHUNK_COLS:
                chunks.append((cur[0][1], cur[-1][2], cur))
                cur, cur_n = [], 0
            cur.append((s, p0, p1))
            cur_n += n
        if cur:
            chunks.append((cur[0][1], cur[-1][2], cur))
        blocks.append({"tiles": tiles, "pieces": pieces, "chunks": chunks})
    return {
        "blocks": blocks,
        "owner": owner,
        "tile_ncols": tile_ncols,
        "total_cols": c,
    }


def build_plan(edge_index):
    src0 = np.asarray(edge_index[0], dtype=np.int64)
    dst0 = np.asarray(edge_index[1], dtype=np.int64)

    deg = np.bincount(dst0, minlength=N).astype(np.int64) + 1  # + self loop

    order = np.argsort(-deg, kind="stable")
    new_id = np.full(N, -1, dtype=np.int64)
    old_of_new = np.full(NPAD, -1, dtype=np.int64)
    n_data_tiles = (N + P - 1) // P  # 782
    for t in range(n_data_tiles):
        core = t % NCORES
        pos = t // NCORES
        rows = order[t * P : (t + 1) * P]
        base = core * SHARD + pos * P
        new_id[rows] = base + np.arange(len(rows))
        old_of_new[base : base + len(rows)] = rows

    real_ids = np.flatnonzero(old_of_new >= 0).astype(np.int64)
    nsrc = np.concatenate([new_id[src0], real_ids])  # self-loops appended
    ndst = np.concatenate([new_id[dst0], real_ids])

    wtp = np.asarray(WT)
    woff = np.asarray(WOFF[:4])
    # source window + window-local table row (int16-safe, < 25600)
    sc, sr = nsrc // SHARD, nsrc % SHARD
    si, sp = sr // P, sr % P
    sw = WINDOW_OF_TILE[si]
    slocal = sc * (wtp[sw] * P) + (si - woff[sw]) * P + sp
    # destination coords
    dc, dr = ndst // SHARD, ndst % SHARD
    di, dp = dr // P, dr % P

    # per (core, tile, window) counts -> uniform column counts (max of cores)
    cnt = np.zeros((NCORES, TILES_PER_CORE, 4), np.int64)
    np.add.at(cnt, (dc, di, sw), 1)
    ncols_tw = -(-cnt.max(axis=0) // P)            # [98, 4]

    colspec = []
    gstart = np.full((TILES_PER_CORE, 4), -1, np.int64)
    c = 0
    for b0 in range(0, TILES_PER_CORE, BLK):
        bw = []
        for s in range(4):
            lst = []
            for i in range(b0, min(b0 + BLK, TILES_PER_CORE)):
                nc_ = int(ncols_tw[i, s])
                if nc_:
                    lst.append((i, nc_))
                    gstart[i, s] = c
                    c += nc_
            bw.append(tuple(lst))
        colspec.append(tuple(bw))
    colspec = tuple(colspec)
    layout = layout_from_colspec(colspec)
    total_cols = layout["total_cols"]

    # rank of each edge within its (core, tile, window) group
    gkey = (dc * TILES_PER_CORE + di) * 4 + sw
    o = np.argsort(gkey, kind="stable")
    gs = gkey[o]
    change = np.r_[True, gs[1:] != gs[:-1]]
    grp_starts = np.flatnonzero(change)
    sizes = np.diff(np.r_[grp_starts, len(gs)])
    within = np.arange(len(gs)) - np.repeat(grp_starts, sizes)
    rank = np.empty(len(gs), np.int64)
    rank[o] = within

    colg = gstart[di, sw] + rank // P
    slot = rank % P
    dst_ids = np.full((NCORES, total_cols, P), PAD_DST, np.int16)
    srcl = np.zeros((NCORES, total_cols, P), np.int32)
    dst_ids[dc, colg, slot] = dp
    srcl[dc, colg, slot] = slocal

    # wrapped int16 indices, piece by piece (idx j of a piece lives at
    # partition j%16, column j//16, replicated across the 8 Q7 cores)
    idx16 = np.zeros((NCORES, 16, 8 * total_cols), np.int16)
    for blk in layout["blocks"]:
        for (s, c0, c1) in blk["pieces"]:
            flat = srcl[:, c0:c1, :].reshape(NCORES, -1)
            idx16[:, :, c0 * 8 : c1 * 8] = (
                flat.reshape(NCORES, -1, 16).transpose(0, 2, 1).astype(np.int16)
            )
    idx_all = np.tile(idx16, (1, 8, 1))            # [NCORES, 128, 8*total_cols]

    dst_f16 = dst_ids.transpose(0, 2, 1).astype(np.float16)  # [NCORES, P, cols]

    dinv = np.zeros(NPAD, dtype=np.float64)
    real = old_of_new >= 0
    dinv[real] = 1.0 / np.sqrt(deg[old_of_new[real]].astype(np.float64))
    dinv = dinv.astype(np.float32)
    rdinv = np.zeros(NPAD, dtype=np.float64)
    rdinv[real] = np.sqrt(deg[old_of_new[real]].astype(np.float64))
    rdinv = rdinv.astype(np.float16)
    dinv_cols = [
        dinv[c * SHARD : (c + 1) * SHARD].reshape(TILES_PER_CORE, P).T.copy()
        for c in range(NCORES)
    ]
    rdinv_flat = [
        rdinv[c * SHARD : (c + 1) * SHARD].reshape(1, SHARD).copy()
        for c in range(NCORES)
    ]
    return {
        "old_of_new": old_of_new,
        "colspec": colspec,
        "idx_all": idx_all,
        "dst_f16": dst_f16,
        "dinv_cols": dinv_cols,
        "rdinv_flat": rdinv_flat,
    }


# ---------------------------------------------------------------- program ---


def build_program(colspec, reps=1, skip=()):
    nc = bass.Bass("TRN2", target_bir_lowering=False, debug=False,
                   enable_asserts=True, num_devices=NCORES,
                   num_swdge_queues=NQ)
    layout = layout_from_colspec(colspec)
    blocks = layout["blocks"]
    owner = layout["owner"]
    tile_ncols = layout["tile_ncols"]
    total_cols = layout["total_cols"]

    xT_s = nc.dram_tensor("xT_shard", [P, SHARD], F16, kind="ExternalInput").ap()
    idx = nc.dram_tensor("idx", [P, 8 * total_cols], mybir.dt.int16,
                         kind="ExternalInput").ap()
    dstid = nc.dram_tensor("dstid", [P, total_cols], F16, kind="ExternalInput").ap()
    iota_in = nc.dram_tensor("iota_in", [P, P], F16, kind="ExternalInput").ap()
    dinv_c = nc.dram_tensor("dinv_cols", [P, TILES_PER_CORE], F32,
                            kind="ExternalInput").ap()
    rdinv_c = nc.dram_tensor("rdinv_flat", [1, SHARD], F16,
                             kind="ExternalInput").ap()
    W1e = nc.dram_tensor("W1e", [F1, F1], F16, kind="ExternalInput").ap()
    W2e = nc.dram_tensor("W2e", [F1, F2], F16, kind="ExternalInput").ap()
    Wm = nc.dram_tensor("Wm", [F2, FZ], F16, kind="ExternalInput").ap()
    W1d = nc.dram_tensor("W1d", [FZ, F1], F16, kind="ExternalInput").ap()
    W2d = nc.dram_tensor("W2d", [F1, FO], F16, kind="ExternalInput").ap()
    NB = F1 + F2 + FZ + F1 + FO + 2 * FZ
    biases = nc.dram_tensor("biases", [P, NB], F32, kind="ExternalInput").ap()
    brow = nc.dram_tensor("biasrow", [1, NB], F16, kind="ExternalInput").ap()
    out_t = nc.dram_tensor("out", [SHARD, FO], F32, kind="ExternalOutput").ap()

    bounce1 = nc.dram_tensor("bounce1", [SHARD, F1], F16).ap()
    bounce2 = nc.dram_tensor("bounce2", [SHARD, F1], F16).ap()
    bounce3 = nc.dram_tensor("bounce3", [SHARD, F1], F16).ap()
    bounce4 = nc.dram_tensor("bounce4", [SHARD, FO], F16).ap()
    t1 = nc.dram_tensor("t1", [NPAD, F1], F16, addr_space="Shared").ap()
    t2 = nc.dram_tensor("t2", [NPAD, F1], F16, addr_space="Shared").ap()
    t3 = nc.dram_tensor("t3", [NPAD, F1], F16, addr_space="Shared").ap()
    t4 = nc.dram_tensor("t4", [NPAD, FO], F16, addr_space="Shared").ap()

    rg = [list(range(NCORES))]
    OB1, OB2, OBM, OB1D, OB2D = 0, F1, F1 + F2, F1 + F2 + FZ, F1 + F2 + FZ + F1
    OLNW = OB2D + FO
    OLNB = OLNW + FZ

    with CompatTileContext(nc) as tc:
        with (
            tc.tile_pool(name="const", bufs=1) as constp,
            tc.tile_pool(name="stg", bufs=2) as stgp,
            tc.tile_pool(name="sel", bufs=2) as selp,
            tc.tile_pool(name="work", bufs=3) as workp,
            tc.tile_pool(name="psum", bufs=2, space="PSUM") as psump,
        ):
            nc.gpsimd.load_library(library_config.mlp)
            ident16 = constp.tile([P, P], F16)
            make_identity(nc, ident16[:])
            idx_t = constp.tile([P, 8 * total_cols], mybir.dt.int16)
            nc.sync.dma_start(out=idx_t[:], in_=idx[:])
            dst_t = constp.tile([P, total_cols], F16)
            nc.sync.dma_start(out=dst_t[:], in_=dstid[:])
            iota_t = constp.tile([P, P], F16)
            nc.sync.dma_start(out=iota_t[:], in_=iota_in[:])
            dinv_t = constp.tile([P, TILES_PER_CORE], F32)
            nc.sync.dma_start(out=dinv_t[:], in_=dinv_c[:])
            # transposed sqrt(deg) row: rdinvT_t[0, i*P+p] = sqrt(deg)
            # of tile i's node p; the 1-row lhsT of the rank-1 bias matmul
            # that folds "+ b" into each tile's PSUM accumulation (acc gets
            # rdinv*b, the epilogue's dinv scale turns it into + b).
            rdinvT_t = constp.tile([1, SHARD], F16)
            nc.sync.dma_start(out=rdinvT_t[:], in_=rdinv_c[:])
            brow_t = constp.tile([1, NB], F16)
            nc.sync.dma_start(out=brow_t[:], in_=brow[:])
            xT_t = constp.tile([P, SHARD], F16)
            for xc in range(0, TILES_PER_CORE, 14):
                hi = min(xc + 14, TILES_PER_CORE)
                nc.sync.dma_start(out=xT_t[:, xc * P : hi * P],
                                  in_=xT_s[:, xc * P : hi * P])
            w1e_t = constp.tile([F1, F1], F16)
            nc.sync.dma_start(out=w1e_t[:], in_=W1e[:])
            w2e_t = constp.tile([F1, F2], F16)
            nc.sync.dma_start(out=w2e_t[:], in_=W2e[:])
            wm_t = constp.tile([F2, FZ], F16)
            nc.sync.dma_start(out=wm_t[:], in_=Wm[:])
            w1d_t = constp.tile([FZ, F1], F16)
            nc.sync.dma_start(out=w1d_t[:], in_=W1d[:])
            w2d_t = constp.tile([F1, FO], F16)
            nc.sync.dma_start(out=w2d_t[:], in_=W2d[:])
            bias_t = constp.tile([P, NB], F32)
            nc.sync.dma_start(out=bias_t[:], in_=biases[:])
            inv5_t = constp.tile([P, 1], F32)
            nc.gpsimd.memset(inv5_t[:], 1.0 / FZ)
            eps_t = constp.tile([P, 1], F32)
            nc.gpsimd.memset(eps_t[:], EPS)
            # batched-bottleneck buffers: one 5-wide column block per tile
            zm_all = constp.tile([P, TILES_PER_CORE * FZ], F32)
            zb = constp.tile([P, TILES_PER_CORE * FZ], F32)
            diff = constp.tile([P, TILES_PER_CORE * FZ], F32)

            def ag_chunk(bounce, t, s):
                if "collective" in skip:
                    return
                nc.gpsimd.collective_compute(
                    "AllGather", mybir.AluOpType.bypass, replica_groups=rg,
                    ins=[bounce[WOFF[s] * P : WOFF[s + 1] * P, :]],
                    outs=[t[WBASE[s] : WBASE[s] + WROWS[s], :]])

            def produce(i, act16, w_t, fout, dst_bounce):
                """bounce row tile i: dinv * (act @ W), zero-padded to 128."""
                tr = psump.tile([F1, P], F16, tag="tr")
                nc.tensor.transpose(out=tr[:], in_=act16[:], identity=ident16[:])
                trs = workp.tile([F1, P], F16, tag="trs")
                nc.vector.tensor_copy(out=trs[:], in_=tr[:])
                mm = psump.tile([P, fout], F32, tag="pmm")
                nc.tensor.matmul(mm[:], lhsT=trs[:], rhs=w_t[:, :fout],
                                 start=True, stop=True)
                ms = workp.tile([P, F1], F16, tag="ms")
                if fout < F1:
                    nc.vector.memset(ms[:, fout:], 0.0)
                nc.scalar.activation(ms[:, :fout], mm[:], AF.Copy,
                                     scale=dinv_t[:, i : i + 1])
                nc.sync.dma_start(out=dst_bounce[i * P : (i + 1) * P, :], in_=ms[:])

            def epilogue(i, acc_ap, fout, relu):
                """dinv*acc -> fp16 (relu) or fp32 tile.  The bias is already
                inside acc as rdinv*b (rank-1 matmul), so one activation
                suffices: out = f(dinv*(sum + rdinv*b)) = f(dinv*sum + b)."""
                if relu:
                    e = workp.tile([P, fout], F16, tag="epi")
                    nc.scalar.activation(e[:], acc_ap, AF.Relu,
                                         scale=dinv_t[:, i : i + 1])
                else:
                    e = workp.tile([P, fout], F32, tag="epi")
                    nc.scalar.activation(e[:], acc_ap, AF.Copy,
                                         scale=dinv_t[:, i : i + 1])
                return e

            nidx_regs = {}

            def nidx_reg(v):
                if v not in nidx_regs:
                    nidx_regs[v] = nc.gpsimd.to_reg(v)
                return nidx_regs[v]

            piece_seq = [0]  # round-robin SWDGE queue assignment

            def agg_layer(table, finish, hooks, bias_off, bias_w, skip=()):
                """Gather + S-matmul aggregation over all 13 blocks.
                finish(i, acc_slice) consumes tile i's [P, 128] fp32 sum
                (which includes rdinv*b via a rank-1 bias matmul per tile).
                hooks[bi]() fires after block bi's finishes (used to launch
                the next layer's AllGather chunks mid-aggregation).
                skip: ablation flags ("gather", "select", "matmul")."""
                for bi, blk in enumerate(blocks):
                    tiles = blk["tiles"]
                    banks = [tiles[k : k + 4] for k in range(0, len(tiles), 4)]
                    bank_of = {}
                    slice_of = {}
                    for bk, bt in enumerate(banks):
                        for sl, i in enumerate(bt):
                            bank_of[i] = bk
                            slice_of[i] = sl
                    accs = []
                    for bk, bt in enumerate(banks):
                        a = psump.tile([P, len(bt) * F1], F32, tag=f"agg{bk}",
                                       name=f"agg{bk}_{bi}")
                        accs.append(a)
                    bank_seen = [0] * len(banks)
                    bank_tot = [sum(tile_ncols[i] for i in bt) + len(bt)
                                for bt in banks]
                    if "matmul" in skip:
                        bank_tot = [len(bt) for bt in banks]
                    # rank-1 bias matmuls first: acc[p, :] += rdinv[p]*b
                    for i in tiles:
                        bk, sl = bank_of[i], slice_of[i]
                        nc.tensor.matmul(
                            accs[bk][:, sl * F1 : sl * F1 + bias_w],
                            lhsT=rdinvT_t[:, i * P : (i + 1) * P],
                            rhs=brow_t[:, bias_off : bias_off + bias_w],
                            start=(bank_seen[bk] == 0),
                            stop=(bank_seen[bk] == bank_tot[bk] - 1),
                        )
                        bank_seen[bk] += 1
                    for (c0, c1, pcs) in blk["chunks"]:
                        ncc = c1 - c0
                        stg = None
                        if "gather" not in skip or "matmul" not in skip:
                            stg = stgp.tile([P, ncc * F1], F16, tag="stg")
                        if "gather" not in skip:
                            for (s, pc0, pc1) in pcs:
                                nidx = (pc1 - pc0) * P
                                nc.gpsimd.dma_gather(
                                    stg[:, (pc0 - c0) * F1 : (pc1 - c0) * F1]
                                    .rearrange("p (c e) -> p c e", e=F1),
                                    table[WBASE[s] : WBASE[s] + WROWS[s], :],
                                    idx_t[:, pc0 * 8 : pc1 * 8],
                                    nidx, nidx_reg(nidx), F1,
                                    queue_num=piece_seq[0] % NQ,
                                )
                                piece_seq[0] += 1
                        S = None
                        if "select" not in skip or "matmul" not in skip:
                            S = selp.tile([P, ncc * P], F16, tag="S")
                        if "select" not in skip:
                            nc.vector.tensor_tensor(
                                out=S[:].rearrange("p (c q) -> p c q", q=P),
                                in0=dst_t[:, c0:c1].unsqueeze(2)
                                    .broadcast_to([P, ncc, P]),
                                in1=iota_t[:].unsqueeze(1)
                                    .broadcast_to([P, ncc, P]),
                                op=mybir.AluOpType.is_equal,
                            )
                        for c in range(c0, c1):
                            i = owner[c]
                            bk = bank_of[i]
                            sl = slice_of[i]
                            if "matmul" in skip:
                                bank_seen[bk] += 1
                                continue
                            nc.tensor.matmul(
                                accs[bk][:, sl * F1 : (sl + 1) * F1],
                                lhsT=S[:, (c - c0) * P : (c - c0 + 1) * P],
                                rhs=stg[:, (c - c0) * F1 : (c - c0 + 1) * F1],
                                start=(bank_seen[bk] == 0),
                                stop=(bank_seen[bk] == bank_tot[bk] - 1),
                            )
                            bank_seen[bk] += 1
                    for i in tiles:
                        bk, sl = bank_of[i], slice_of[i]
                        finish(i, accs[bk][:, sl * F1 : (sl + 1) * F1])
                    if bi in hooks:
                        hooks[bi]()

            def ln_chunk(s):
                """Batched bottleneck over window s's tiles: +bm, layernorm,
                then per tile q = dinv*(LN(z) @ W1d) -> bounce3; AG3 chunk."""
                T_ = WT[s]
                lo = WOFF[s]
                def r3(ap):
                    return ap.rearrange("p (t z) -> p t z", z=FZ)
                zmv = zm_all[:, lo * FZ : (lo + T_) * FZ]
                zbv = zb[:, lo * FZ : (lo + T_) * FZ]
                dfv = diff[:, lo * FZ : (lo + T_) * FZ]
                bm3 = bias_t[:, OBM : OBM + FZ].unsqueeze(1).broadcast_to([P, T_, FZ])
                nc.vector.tensor_tensor(
                    out=r3(zbv), in0=r3(zmv), in1=bm3, op=mybir.AluOpType.add)
                musum = workp.tile([P, T_], F32, tag="musum")
                nc.vector.reduce_sum(musum[:], r3(zbv), axis=mybir.AxisListType.X)
                mu = workp.tile([P, T_], F32, tag="mu")
                nc.vector.tensor_mul(out=mu[:], in0=musum[:],
                                     in1=inv5_t[:].broadcast_to([P, T_]))
                nc.vector.tensor_tensor(
                    out=r3(dfv), in0=r3(zbv),
                    in1=mu[:].unsqueeze(2).broadcast_to([P, T_, FZ]),
                    op=mybir.AluOpType.subtract)
                sq = workp.tile([P, T_ * FZ], F32, tag="sq")
                nc.vector.tensor_mul(out=sq[:], in0=dfv, in1=dfv)
                varsum = workp.tile([P, T_], F32, tag="varsum")
                nc.vector.reduce_sum(
                    varsum[:], sq[:].rearrange("p (t z) -> p t z", z=FZ),
                    axis=mybir.AxisListType.X)
                var = workp.tile([P, T_], F32, tag="var")
                nc.vector.tensor_mul(out=var[:], in0=varsum[:],
                                     in1=inv5_t[:].broadcast_to([P, T_]))
                vare = workp.tile([P, T_], F32, tag="vare")
                nc.vector.tensor_add(out=vare[:], in0=var[:],
                                     in1=eps_t[:].broadcast_to([P, T_]))
                sd = workp.tile([P, T_], F32, tag="sd")
                nc.scalar.activation(sd[:], vare[:], AF.Sqrt)
                rinv = workp.tile([P, T_], F32, tag="rinv")
                nc.vector.reciprocal(rinv[:], sd[:])
                zn = workp.tile([P, T_ * FZ], F32, tag="zn")
                nc.vector.tensor_tensor(
                    out=r3(zn[:]), in0=r3(dfv),
                    in1=rinv[:].unsqueeze(2).broadcast_to([P, T_, FZ]),
                    op=mybir.AluOpType.mult)
                zw = workp.tile([P, T_ * FZ], F32, tag="zw")
                nc.vector.tensor_tensor(
                    out=r3(zw[:]), in0=r3(zn[:]),
                    in1=bias_t[:, OLNW : OLNW + FZ].unsqueeze(1)
                        .broadcast_to([P, T_, FZ]),
                    op=mybir.AluOpType.mult)
                zl = workp.tile([P, T_ * FZ], F16, tag="zl")
                nc.vector.tensor_tensor(
                    out=r3(zl[:]), in0=r3(zw[:]),
                    in1=bias_t[:, OLNB : OLNB + FZ].unsqueeze(1)
                        .broadcast_to([P, T_, FZ]),
                    op=mybir.AluOpType.add)
                for k in range(T_):
                    i = lo + k
                    tr3 = psump.tile([FZ, P], F16, tag="tr")
                    nc.tensor.transpose(out=tr3[:], in_=zl[:, k * FZ:(k + 1) * FZ],
                                        identity=ident16[:])
                    zT = workp.tile([FZ, P], F16, tag="trs")
                    nc.vector.tensor_copy(out=zT[:], in_=tr3[:])
                    qp = psump.tile([P, F1], F32, tag="pmm")
                    nc.tensor.matmul(qp[:], lhsT=zT[:], rhs=w1d_t[:],
                                     start=True, stop=True)
                    q16 = workp.tile([P, F1], F16, tag="ms")
                    nc.scalar.activation(q16[:], qp[:], AF.Copy,
                                         scale=dinv_t[:, i : i + 1])
                    nc.sync.dma_start(out=bounce3[i * P : (i + 1) * P, :],
                                      in_=q16[:])
                ag_chunk(bounce3, t3, s)

            for _rep in range(reps):
                # ---- L1 produce: xT is resident, one matmul per tile
                for i in range(TILES_PER_CORE):
                    for s in range(3):
                        if i == WOFF[s + 1]:
                            ag_chunk(bounce1, t1, s)
                    mm = psump.tile([P, F1], F32, tag="pmm")
                    nc.tensor.matmul(
                        mm[:], lhsT=xT_t[:, i * P : (i + 1) * P], rhs=w1e_t[:],
                        start=True, stop=True)
                    ms = workp.tile([P, F1], F16, tag="ms")
                    nc.scalar.activation(ms[:], mm[:], AF.Copy,
                                         scale=dinv_t[:, i : i + 1])
                    nc.sync.dma_start(out=bounce1[i * P : (i + 1) * P, :],
                                      in_=ms[:])
                ag_chunk(bounce1, t1, 3)

                # ---- L1 aggregate -> h (relu) -> L2 produce (zero-padded)
                def fin1(i, acc):
                    h16 = epilogue(i, acc, F1, relu=True)
                    produce(i, h16, w2e_t, F2, bounce2)

                agg_layer(t1, fin1, {
                    HOOK_BLOCKS[0]: lambda: ag_chunk(bounce2, t2, 0),
                    HOOK_BLOCKS[1]: lambda: ag_chunk(bounce2, t2, 1),
                    HOOK_BLOCKS[2]: lambda: ag_chunk(bounce2, t2, 2),
                }, OB1, F1, skip)
                ag_chunk(bounce2, t2, 3)

                # ---- L2 aggregate -> z -> (relu z) @ Wm staged per tile
                def fin2(i, acc2):
                    zr16 = epilogue(i, acc2[:, :F2], F2, relu=True)
                    tr2 = psump.tile([F2, P], F16, tag="tr")
                    nc.tensor.transpose(out=tr2[:], in_=zr16[:],
                                        identity=ident16[:])
                    tr2s = workp.tile([F2, P], F16, tag="trs")
                    nc.vector.tensor_copy(out=tr2s[:], in_=tr2[:])
                    zm = psump.tile([P, FZ], F32, tag="pmm")
                    nc.tensor.matmul(zm[:], lhsT=tr2s[:], rhs=wm_t[:],
                                     start=True, stop=True)
                    nc.vector.tensor_copy(
                        out=zm_all[:, i * FZ : (i + 1) * FZ], in_=zm[:])

                agg_layer(t2, fin2, {
                    HOOK_BLOCKS[0]: lambda: ln_chunk(0),
                    HOOK_BLOCKS[1]: lambda: ln_chunk(1),
                    HOOK_BLOCKS[2]: lambda: ln_chunk(2),
                }, OB2, F2, skip)
                ln_chunk(3)

                # ---- L3 aggregate (q = dinv*(LN(z)@W1d) rows) -> d -> L4
                def fin3(i, acc3):
                    d16 = epilogue(i, acc3, F1, relu=True)
                    produce(i, d16, w2d_t, FO, bounce4)

                agg_layer(t3, fin3, {
                    HOOK_BLOCKS[0]: lambda: ag_chunk(bounce4, t4, 0),
                    HOOK_BLOCKS[1]: lambda: ag_chunk(bounce4, t4, 1),
                    HOOK_BLOCKS[2]: lambda: ag_chunk(bounce4, t4, 2),
                }, OB1D, F1, skip)
                ag_chunk(bounce4, t4, 3)

                # ---- L4 aggregate -> output
                def fin4(i, acc4):
                    o = epilogue(i, acc4, FO, relu=False)
                    nc.sync.dma_start(out=out_t[i * P : (i + 1) * P, :], in_=o[:])

                agg_layer(t4, fin4, {}, OB2D, FO, skip)
    return nc


# ------------------------------------------------------------------ kernel --

_CACHE = {}


def kernel(x, edge_index, W1e, b1e, W2e, b2e, Wm, bm, ln_w, ln_b,
           W1d, b1d, W2d, b2d):
    x = np.asarray(x, dtype=np.float32)
    edge_index = np.asarray(edge_index)
    plan = build_plan(edge_index)
    old_of_new = plan["old_of_new"]
    real = old_of_new >= 0

    # pack per-core inputs
    xg = np.zeros((NPAD, F1), np.float16)
    xg[real] = x[old_of_new[real]].astype(np.float16)
    bias_pack = np.zeros((P, F1 + F2 + FZ + F1 + FO + 2 * FZ), np.float32)
    o = 0
    for vec in (b1e, b2e, bm, b1d, b2d, ln_w, ln_b):
        v = np.asarray(vec, np.float32).ravel()
        bias_pack[:, o : o + v.size] = v[None, :]
        o += v.size
    iota = np.tile(np.arange(P, dtype=np.float16)[None, :], (P, 1))

    in_maps = []
    for c in range(NCORES):
        in_maps.append({
            "xT_shard": xg[c * SHARD : (c + 1) * SHARD].T.copy(),
            "idx": plan["idx_all"][c],
            "dstid": plan["dst_f16"][c],
            "iota_in": iota,
            "dinv_cols": plan["dinv_cols"][c],
            "rdinv_flat": plan["rdinv_flat"][c],
            "biasrow": bias_pack[:1].astype(np.float16),
            "W1e": np.asarray(W1e, np.float16),
            "W2e": np.asarray(W2e, np.float16),
            "Wm": np.asarray(Wm, np.float16),
            "W1d": np.asarray(W1d, np.float16),
            "W2d": np.asarray(W2d, np.float16),
            "biases": bias_pack,
        })

    key = plan["colspec"]
    if key not in _CACHE:
        nc = build_program(plan["colspec"])
        _CACHE[key] = SpmdRunner(nc)
    runner = _CACHE[key]
    runner.stage(in_maps)
    res = runner.results(runner.run())

    out_new = np.concatenate([res[c]["out"] for c in range(NCORES)], axis=0)
    out = np.zeros((N, FO), np.float32)
    out[old_of_new[real]] = out_new[real]
    return out


# revision 25
# speedup vs baseline: 1.0149x; 1.0149x over previous
"""GCN autoencoder forward pass on 8 Trainium2 NeuronCores (Bass/Tile).

Strategy (graph/data parallel per the sharding hint):
  - Nodes permuted by in-degree, dealt to 8 cores tile-round-robin
    (128-node tiles -> near-uniform per-tile degree, balanced edges).
  - Per conv layer: per-core matmul produces table rows m = dinv*(act @ W)
    in fp16; the shard tables are AllGathered (halo exchange) into a
    Shared-scratchpad full table in FOUR window chunks (windows = 25/25/
    24/24 tile ranges) so each chunk overlaps the producing phase and the
    consuming gathers.
  - Aggregation is gather + matmul: per block of 8 destination tiles, the
    block's in-edge messages are staged into SBUF with a handful of
    SWDGE dma_gather instructions (dense 128-slot columns; indices are
    window-local int16), a 0/1 selection matrix S[slot, dst] is built on
    the DVE with one is_equal per chunk, and TensorE matmuls
    acc[dst,:] += S_c^T @ stage_c accumulate each tile's messages in
    PSUM (four tiles share one 2KB bank as slices of one accumulation
    group - only the globally-first matmul uses start=True).  This
    replaces per-edge-wave indirect DMAs (~1us SWDGE fixed cost each)
    and the DVE fold tree of the previous revision.
  - t2 (64-wide) rows are zero-padded to 128 so all tables gather with
    elem_size=128; t3 holds q = dinv*(LN(z) @ W1d) (the 5->128 matmul is
    hoisted BEFORE aggregation - matmul and scatter-add commute), so the
    decoder aggregates 128-wide as well.
  - The 5-wide bottleneck (relu->linear->layernorm) is computed batched
    per window chunk (amortizes DVE/Scalar instruction overheads).
  - Per-layer biases ride a rank-1 matmul into each tile's PSUM group
    (lhsT = 1-row sqrt(deg) slice, rhs = bias row), so the epilogue is a
    single Scalar activation (relu/copy with per-partition dinv scale).
  - Tables, bounces, weights and matmul operands are fp16 (rel err
    ~8e-4, tolerance 2e-2); aggregation accumulates fp32 in PSUM.

HW notes (measured on axon-tunneled trn2): dma_gather hangs the device
above 1024 descriptors per instruction (SWDGE ring), hence PIECE_COLS=8;
4 SWDGE queues give ~4x gather throughput (~1.3-1.9 ns/descriptor at
256B rows); multi-index InstDMACopy (indirect_dma_start with a >1-column
offset AP) does NOT implement gather semantics on HW - only dma_gather
does.  Measured ~3.1 ms/rep vs 5.4 ms for the per-edge-wave + DVE-fold
predecessor.

Self-contained: includes the walrus sync-wait compat shim and a PJRT
runner (axon) replicating bass2jax.run_bass_via_pjrt.
"""

import sys

for _p in ("/opt/trn_rl_repo",):
    if _p not in sys.path:
        sys.path.insert(0, _p)

import numpy as np

import concourse.bass as bass
import concourse.mybir as mybir
import concourse.tile as tile
from concourse import library_config
from concourse.masks import make_identity

P = 128
NCORES = 8
N = 100000
TILES_PER_CORE = 98
SHARD = TILES_PER_CORE * P          # 12544
NPAD = NCORES * SHARD               # 100352
F1, F2, FZ, FO = 128, 64, 5, 128
EPS = 1e-5
AF = mybir.ActivationFunctionType
F16 = mybir.dt.float16
F32 = mybir.dt.float32

BLK = 8                              # dst tiles per aggregation block
# AllGather window sizes (tiles); each window must stay under 32 tiles
# (32768 rows) so gather indices fit int16.
WT = (25, 25, 24, 24)
WOFF = (0, 25, 50, 74, 98)
WROWS = tuple(NCORES * t * P for t in WT)      # rows per window
WBASE = (0, 25600, 51200, 75776)               # table row base per window
HOOK_BLOCKS = tuple((WOFF[s + 1] - 1) // BLK for s in range(3))  # (3, 6, 9)
CHUNK_COLS = 80                      # stage/S chunk size (columns)
PIECE_COLS = 8                       # columns per dma_gather (1024 descs =
                                     # the hard SWDGE ring capacity)
NQ = 4                               # SWDGE queues (gathers round-robin)
PAD_DST = 255                        # dst id for padding slots

# ---------------------------------------------------------------- compat ----

MAX_WAITS = 1


def _split_sync_waits(nc, max_waits=MAX_WAITS):
    """This container's walrus rejects >1 sync wait per instruction; move
    excess waits onto same-engine NOPs placed just before the offender."""
    for fn in nc.m.functions:
        for bb in fn.blocks:
            out = []
            for inst in bb.instructions:
                si = inst.sync_info
                if si is not None and si.on_wait and len(si.on_wait) > max_waits:
                    waits = list(si.on_wait)
                    head, tail = waits[:-max_waits], waits[-max_waits:]
                    for i in range(0, len(head), max_waits):
                        out.append(
                            mybir.InstNoOp(
                                name=f"{inst.name}-ws{i}",
                                engine=inst.engine,
                                bass_nofuse=True,
                                sync_info=mybir.SyncInfo(
                                    on_wait=head[i : i + max_waits], on_update=[]
                                ),
                            )
                        )
                    si.on_wait = tail
                out.append(inst)
            bb.instructions[:] = out


class CompatTileContext(tile.TileContext):
    def __exit__(self, *args):
        ret = super().__exit__(*args)
        _split_sync_waits(self.nc)
        return ret


# ---------------------------------------------------------------- runner ----


class SpmdRunner:
    def __init__(self, nc, n_cores=NCORES):
        import jax
        from jax.sharding import Mesh, PartitionSpec, NamedSharding
        from jax.experimental.shard_map import shard_map
        from concourse import bass2jax
        from concourse.bass2jax import _bass_exec_p, install_neuronx_cc_hook

        install_neuronx_cc_hook()
        mybir.codegen_inst_isa_subclasses(nc)
        self.jax = jax
        self.nc = nc
        self.n_cores = n_cores
        partition_name = (
            nc.partition_id_tensor.name if nc.partition_id_tensor else None
        )

        in_names, out_names, out_avals, zero_outs = [], [], [], []
        for alloc in nc.m.functions[0].allocations:
            if not isinstance(alloc, mybir.MemoryLocationSet):
                continue
            name = alloc.memorylocations[0].name
            if alloc.kind == "ExternalInput":
                if name != partition_name:
                    in_names.append(name)
            elif alloc.kind == "ExternalOutput":
                out_names.append(name)
                shape = tuple(alloc.tensor_shape)
                dtype = mybir.dt.np(alloc.dtype)
                out_avals.append(jax.core.ShapedArray(shape, dtype))
                zero_outs.append(np.zeros(shape, dtype))
        self.in_names = in_names
        self.out_names = out_names
        self.out_avals = out_avals
        self.zero_outs = zero_outs
        n_params = len(in_names)
        all_in_names = in_names + out_names
        if partition_name is not None:
            all_in_names = all_in_names + [partition_name]

        def _body(*args):
            operands = list(args)
            if partition_name is not None:
                operands.append(bass2jax.partition_id_tensor())
            outs = _bass_exec_p.bind(
                *operands,
                out_avals=tuple(out_avals),
                in_names=tuple(all_in_names),
                out_names=tuple(out_names),
                lowering_input_output_aliases=(),
                sim_require_finite=True,
                sim_require_nnan=True,
                nc=nc,
            )
            return tuple(outs)

        devices = jax.devices()[:n_cores]
        self.mesh = Mesh(np.asarray(devices), ("core",))
        in_specs = (PartitionSpec("core"),) * (n_params + len(out_names))
        out_specs = (PartitionSpec("core"),) * len(out_names)
        self.sharding = NamedSharding(self.mesh, PartitionSpec("core"))
        self.fn = jax.jit(
            shard_map(_body, mesh=self.mesh, in_specs=in_specs,
                      out_specs=out_specs, check_rep=False),
            keep_unused=True,
        )
        self._dev_args = None

    def stage(self, in_maps):
        self._staged_in_maps = in_maps
        n = self.n_cores
        concat = [
            np.concatenate([np.asarray(in_maps[c][name]) for c in range(n)], axis=0)
            for name in self.in_names
        ]
        concat += [
            np.zeros((n * z.shape[0], *z.shape[1:]), z.dtype) for z in self.zero_outs
        ]
        self._dev_args = [self.jax.device_put(a, self.sharding) for a in concat]
        for a in self._dev_args:
            a.block_until_ready()

    def run(self):
        outs = self.fn(*self._dev_args)
        self.jax.block_until_ready(outs)
        return outs

    def results(self, outs):
        res = []
        for c in range(self.n_cores):
            d = {}
            for i, name in enumerate(self.out_names):
                full = np.asarray(outs[i])
                d[name] = full.reshape(self.n_cores, *self.out_avals[i].shape)[c]
            res.append(d)
        return res


# ------------------------------------------------------------------ plan ----

WINDOW_OF_TILE = np.concatenate(
    [np.full(WT[s], s, dtype=np.int64) for s in range(4)])


def layout_from_colspec(colspec):
    """colspec: per block, per window, list of (tile, ncols).  Returns the
    global column layout shared by the host packer and the program builder:
      blocks: list of dicts with
        tiles: list of tile ids in the block
        pieces: [(s, c0, c1)] gather instructions (global col ranges)
        chunks: [(c0, c1, [(s, pc0, pc1), ...])] stage/S granularity
      owner[c]: tile owning global column c
      tile_ncols[i]: total columns of tile i
      total_cols
    """
    owner = []
    blocks = []
    tile_ncols = {}
    c = 0
    for bi, bw in enumerate(colspec):
        b0 = bi * BLK
        tiles = list(range(b0, min(b0 + BLK, TILES_PER_CORE)))
        pieces = []
        for s in range(4):
            g0 = c
            for (i, ncols) in bw[s]:
                owner.extend([i] * ncols)
                tile_ncols[i] = tile_ncols.get(i, 0) + ncols
                c += ncols
            # split the window group into <= PIECE_COLS gather pieces
            p0 = g0
            while p0 < c:
                p1 = min(p0 + PIECE_COLS, c)
                pieces.append((s, p0, p1))
                p0 = p1
        # greedy merge consecutive pieces into chunks of <= CHUNK_COLS
        chunks = []
        cur = []
        cur_n = 0
        for (s, p0, p1) in pieces:
            n = p1 - p0
            if cur and cur_n + n > CHUNK_COLS:
                chunks.append((cur[0][1], cur[-1][2], cur))
                cur, cur_n = [], 0
            cur.append((s, p0, p1))
            cur_n += n
        if cur:
            chunks.append((cur[0][1], cur[-1][2], cur))
        blocks.append({"tiles": tiles, "pieces": pieces, "chunks": chunks})
    return {
        "blocks": blocks,
        "owner": owner,
        "tile_ncols": tile_ncols,
        "total_cols": c,
    }


def build_plan(edge_index):
    src0 = np.asarray(edge_index[0], dtype=np.int64)
    dst0 = np.asarray(edge_index[1], dtype=np.int64)

    deg = np.bincount(dst0, minlength=N).astype(np.int64) + 1  # + self loop

    order = np.argsort(-deg, kind="stable")
    new_id = np.full(N, -1, dtype=np.int64)
    old_of_new = np.full(NPAD, -1, dtype=np.int64)
    n_data_tiles = (N + P - 1) // P  # 782
    for t in range(n_data_tiles):
        core = t % NCORES
        pos = t // NCORES
        rows = order[t * P : (t + 1) * P]
        base = core * SHARD + pos * P
        new_id[rows] = base + np.arange(len(rows))
        old_of_new[base : base + len(rows)] = rows

    real_ids = np.flatnonzero(old_of_new >= 0).astype(np.int64)
    nsrc = np.concatenate([new_id[src0], real_ids])  # self-loops appended
    ndst = np.concatenate([new_id[dst0], real_ids])

    wtp = np.asarray(WT)
    woff = np.asarray(WOFF[:4])
    # source window + window-local table row (int16-safe, < 25600)
    sc, sr = nsrc // SHARD, nsrc % SHARD
    si, sp = sr // P, sr % P
    sw = WINDOW_OF_TILE[si]
    slocal = sc * (wtp[sw] * P) + (si - woff[sw]) * P + sp
    # destination coords
    dc, dr = ndst // SHARD, ndst % SHARD
    di, dp = dr // P, dr % P

    # per (core, tile, window) counts -> uniform column counts (max of cores)
    cnt = np.zeros((NCORES, TILES_PER_CORE, 4), np.int64)
    np.add.at(cnt, (dc, di, sw), 1)
    ncols_tw = -(-cnt.max(axis=0) // P)            # [98, 4]

    colspec = []
    gstart = np.full((TILES_PER_CORE, 4), -1, np.int64)
    c = 0
    for b0 in range(0, TILES_PER_CORE, BLK):
        bw = []
        for s in range(4):
            lst = []
            for i in range(b0, min(b0 + BLK, TILES_PER_CORE)):
                nc_ = int(ncols_tw[i, s])
                if nc_:
                    lst.append((i, nc_))
                    gstart[i, s] = c
                    c += nc_
            bw.append(tuple(lst))
        colspec.append(tuple(bw))
    colspec = tuple(colspec)
    layout = layout_from_colspec(colspec)
    total_cols = layout["total_cols"]

    # rank of each edge within its (core, tile, window) group
    gkey = (dc * TILES_PER_CORE + di) * 4 + sw
    o = np.argsort(gkey, kind="stable")
    gs = gkey[o]
    change = np.r_[True, gs[1:] != gs[:-1]]
    grp_starts = np.flatnonzero(change)
    sizes = np.diff(np.r_[grp_starts, len(gs)])
    within = np.arange(len(gs)) - np.repeat(grp_starts, sizes)
    rank = np.empty(len(gs), np.int64)
    rank[o] = within

    colg = gstart[di, sw] + rank // P
    slot = rank % P
    dst_ids = np.full((NCORES, total_cols, P), PAD_DST, np.int16)
    srcl = np.zeros((NCORES, total_cols, P), np.int32)
    dst_ids[dc, colg, slot] = dp
    srcl[dc, colg, slot] = slocal

    # wrapped int16 indices, piece by piece (idx j of a piece lives at
    # partition j%16, column j//16, replicated across the 8 Q7 cores)
    idx16 = np.zeros((NCORES, 16, 8 * total_cols), np.int16)
    for blk in layout["blocks"]:
        for (s, c0, c1) in blk["pieces"]:
            flat = srcl[:, c0:c1, :].reshape(NCORES, -1)
            idx16[:, :, c0 * 8 : c1 * 8] = (
                flat.reshape(NCORES, -1, 16).transpose(0, 2, 1).astype(np.int16)
            )
    idx_all = np.tile(idx16, (1, 8, 1))            # [NCORES, 128, 8*total_cols]

    dst_f16 = dst_ids.transpose(0, 2, 1).astype(np.float16)  # [NCORES, P, cols]

    dinv = np.zeros(NPAD, dtype=np.float64)
    real = old_of_new >= 0
    dinv[real] = 1.0 / np.sqrt(deg[old_of_new[real]].astype(np.float64))
    dinv = dinv.astype(np.float32)
    rdinv = np.zeros(NPAD, dtype=np.float64)
    rdinv[real] = np.sqrt(deg[old_of_new[real]].astype(np.float64))
    rdinv = rdinv.astype(np.float16)
    dinv_cols = [
        dinv[c * SHARD : (c + 1) * SHARD].reshape(TILES_PER_CORE, P).T.copy()
        for c in range(NCORES)
    ]
    rdinv_flat = [
        rdinv[c * SHARD : (c + 1) * SHARD].reshape(1, SHARD).copy()
        for c in range(NCORES)
    ]
    return {
        "old_of_new": old_of_new,
        "colspec": colspec,
        "idx_all": idx_all,
        "dst_f16": dst_f16,
        "dinv_cols": dinv_cols,
        "rdinv_flat": rdinv_flat,
    }


# ---------------------------------------------------------------- program ---


def build_program(colspec, reps=1, skip=()):
    nc = bass.Bass("TRN2", target_bir_lowering=False, debug=False,
                   enable_asserts=True, num_devices=NCORES,
                   num_swdge_queues=NQ)
    layout = layout_from_colspec(colspec)
    blocks = layout["blocks"]
    owner = layout["owner"]
    tile_ncols = layout["tile_ncols"]
    total_cols = layout["total_cols"]

    xT_s = nc.dram_tensor("xT_shard", [P, SHARD], F16, kind="ExternalInput").ap()
    idx = nc.dram_tensor("idx", [P, 8 * total_cols], mybir.dt.int16,
                         kind="ExternalInput").ap()
    dstid = nc.dram_tensor("dstid", [P, total_cols], F16, kind="ExternalInput").ap()
    iota_in = nc.dram_tensor("iota_in", [P, P], F16, kind="ExternalInput").ap()
    dinv_c = nc.dram_tensor("dinv_cols", [P, TILES_PER_CORE], F32,
                            kind="ExternalInput").ap()
    rdinv_c = nc.dram_tensor("rdinv_flat", [1, SHARD], F16,
                             kind="ExternalInput").ap()
    W1e = nc.dram_tensor("W1e", [F1, F1], F16, kind="ExternalInput").ap()
    W2e = nc.dram_tensor("W2e", [F1, F2], F16, kind="ExternalInput").ap()
    Wm = nc.dram_tensor("Wm", [F2, FZ], F16, kind="ExternalInput").ap()
    W1d = nc.dram_tensor("W1d", [FZ, F1], F16, kind="ExternalInput").ap()
    W2d = nc.dram_tensor("W2d", [F1, FO], F16, kind="ExternalInput").ap()
    NB = F1 + F2 + FZ + F1 + FO + 2 * FZ
    biases = nc.dram_tensor("biases", [P, NB], F32, kind="ExternalInput").ap()
    brow = nc.dram_tensor("biasrow", [1, NB], F16, kind="ExternalInput").ap()
    out_t = nc.dram_tensor("out", [SHARD, FO], F32, kind="ExternalOutput").ap()

    bounce1 = nc.dram_tensor("bounce1", [SHARD, F1], F16).ap()
    bounce2 = nc.dram_tensor("bounce2", [SHARD, F1], F16).ap()
    bounce3 = nc.dram_tensor("bounce3", [SHARD, F1], F16).ap()
    bounce4 = nc.dram_tensor("bounce4", [SHARD, FO], F16).ap()
    t1 = nc.dram_tensor("t1", [NPAD, F1], F16, addr_space="Shared").ap()
    t2 = nc.dram_tensor("t2", [NPAD, F1], F16, addr_space="Shared").ap()
    t3 = nc.dram_tensor("t3", [NPAD, F1], F16, addr_space="Shared").ap()
    t4 = nc.dram_tensor("t4", [NPAD, FO], F16, addr_space="Shared").ap()

    rg = [list(range(NCORES))]
    OB1, OB2, OBM, OB1D, OB2D = 0, F1, F1 + F2, F1 + F2 + FZ, F1 + F2 + FZ + F1
    OLNW = OB2D + FO
    OLNB = OLNW + FZ

    with CompatTileContext(nc) as tc:
        with (
            tc.tile_pool(name="const", bufs=1) as constp,
            tc.tile_pool(name="stg", bufs=2) as stgp,
            tc.tile_pool(name="sel", bufs=2) as selp,
            tc.tile_pool(name="work", bufs=3) as workp,
            tc.tile_pool(name="psum", bufs=2, space="PSUM") as psump,
        ):
            nc.gpsimd.load_library(library_config.mlp)
            ident16 = constp.tile([P, P], F16)
            make_identity(nc, ident16[:])
            idx_t = constp.tile([P, 8 * total_cols], mybir.dt.int16)
            nc.sync.dma_start(out=idx_t[:], in_=idx[:])
            dst_t = constp.tile([P, total_cols], F16)
            nc.sync.dma_start(out=dst_t[:], in_=dstid[:])
            iota_t = constp.tile([P, P], F16)
            nc.sync.dma_start(out=iota_t[:], in_=iota_in[:])
            dinv_t = constp.tile([P, TILES_PER_CORE], F32)
            nc.sync.dma_start(out=dinv_t[:], in_=dinv_c[:])
            # transposed sqrt(deg) row: rdinvT_t[0, i*P+p] = sqrt(deg)
            # of tile i's node p; the 1-row lhsT of the rank-1 bias matmul
            # that folds "+ b" into each tile's PSUM accumulation (acc gets
            # rdinv*b, the epilogue's dinv scale turns it into + b).
            rdinvT_t = constp.tile([1, SHARD], F16)
            nc.sync.dma_start(out=rdinvT_t[:], in_=rdinv_c[:])
            brow_t = constp.tile([1, NB], F16)
            nc.sync.dma_start(out=brow_t[:], in_=brow[:])
            xT_t = constp.tile([P, SHARD], F16)
            for xc in range(0, TILES_PER_CORE, 14):
                hi = min(xc + 14, TILES_PER_CORE)
                nc.sync.dma_start(out=xT_t[:, xc * P : hi * P],
                                  in_=xT_s[:, xc * P : hi * P])
            w1e_t = constp.tile([F1, F1], F16)
            nc.sync.dma_start(out=w1e_t[:], in_=W1e[:])
            w2e_t = constp.tile([F1, F2], F16)
            nc.sync.dma_start(out=w2e_t[:], in_=W2e[:])
            wm_t = constp.tile([F2, FZ], F16)
            nc.sync.dma_start(out=wm_t[:], in_=Wm[:])
            w1d_t = constp.tile([FZ, F1], F16)
            nc.sync.dma_start(out=w1d_t[:], in_=W1d[:])
            w2d_t = constp.tile([F1, FO], F16)
            nc.sync.dma_start(out=w2d_t[:], in_=W2d[:])
            bias_t = constp.tile([P, NB], F32)
            nc.sync.dma_start(out=bias_t[:], in_=biases[:])
            inv5_t = constp.tile([P, 1], F32)
            nc.gpsimd.memset(inv5_t[:], 1.0 / FZ)
            eps_t = constp.tile([P, 1], F32)
            nc.gpsimd.memset(eps_t[:], EPS)
            # batched-bottleneck buffers: one 5-wide column block per tile
            zm_all = constp.tile([P, TILES_PER_CORE * FZ], F32)
            zb = constp.tile([P, TILES_PER_CORE * FZ], F32)
            diff = constp.tile([P, TILES_PER_CORE * FZ], F32)

            def ag_chunk(bounce, t, s):
                if "collective" in skip:
                    return
                nc.gpsimd.collective_compute(
                    "AllGather", mybir.AluOpType.bypass, replica_groups=rg,
                    ins=[bounce[WOFF[s] * P : WOFF[s + 1] * P, :]],
                    outs=[t[WBASE[s] : WBASE[s] + WROWS[s], :]])

            def produce(i, act16, w_t, fout, dst_bounce):
                """bounce row tile i: dinv * (act @ W), zero-padded to 128."""
                tr = psump.tile([F1, P], F16, tag="tr")
                nc.tensor.transpose(out=tr[:], in_=act16[:], identity=ident16[:])
                trs = workp.tile([F1, P], F16, tag="trs")
                nc.vector.tensor_copy(out=trs[:], in_=tr[:])
                mm = psump.tile([P, fout], F32, tag="pmm")
                nc.tensor.matmul(mm[:], lhsT=trs[:], rhs=w_t[:, :fout],
                                 start=True, stop=True)
                ms = workp.tile([P, F1], F16, tag="ms")
                if fout < F1:
                    nc.vector.memset(ms[:, fout:], 0.0)
                nc.scalar.activation(ms[:, :fout], mm[:], AF.Copy,
                                     scale=dinv_t[:, i : i + 1])
                nc.sync.dma_start(out=dst_bounce[i * P : (i + 1) * P, :], in_=ms[:])

            def epilogue(i, acc_ap, fout, relu):
                """dinv*acc -> fp16 (relu) or fp32 tile.  The bias is already
                inside acc as rdinv*b (rank-1 matmul), so one activation
                suffices: out = f(dinv*(sum + rdinv*b)) = f(dinv*sum + b)."""
                if relu:
                    e = workp.tile([P, fout], F16, tag="epi")
                    nc.scalar.activation(e[:], acc_ap, AF.Relu,
                                         scale=dinv_t[:, i : i + 1])
                else:
                    e = workp.tile([P, fout], F32, tag="epi")
                    nc.scalar.activation(e[:], acc_ap, AF.Copy,
                                         scale=dinv_t[:, i : i + 1])
                return e

            nidx_regs = {}

            def nidx_reg(v):
                if v not in nidx_regs:
                    nidx_regs[v] = nc.gpsimd.to_reg(v)
                return nidx_regs[v]

            piece_seq = [0]  # round-robin SWDGE queue assignment

            def agg_layer(table, finish, hooks, bias_off, bias_w, skip=()):
                """Gather + S-matmul aggregation over all 13 blocks.
                finish(i, acc_slice) consumes tile i's [P, 128] fp32 sum
                (which includes rdinv*b via a rank-1 bias matmul per tile).
                hooks[bi]() fires after block bi's finishes (used to launch
                the next layer's AllGather chunks mid-aggregation).
                skip: ablation flags ("gather", "select", "matmul")."""
                for bi, blk in enumerate(blocks):
                    tiles = blk["tiles"]
                    banks = [tiles[k : k + 4] for k in range(0, len(tiles), 4)]
                    bank_of = {}
                    slice_of = {}
                    for bk, bt in enumerate(banks):
                        for sl, i in enumerate(bt):
                            bank_of[i] = bk
                            slice_of[i] = sl
                    accs = []
                    for bk, bt in enumerate(banks):
                        a = psump.tile([P, len(bt) * F1], F32, tag=f"agg{bk}",
                                       name=f"agg{bk}_{bi}")
                        accs.append(a)
                    bank_seen = [0] * len(banks)
                    bank_tot = [sum(tile_ncols[i] for i in bt) + len(bt)
                                for bt in banks]
                    if "matmul" in skip:
                        bank_tot = [len(bt) for bt in banks]
                    # rank-1 bias matmuls first: acc[p, :] += rdinv[p]*b
                    for i in tiles:
                        bk, sl = bank_of[i], slice_of[i]
                        nc.tensor.matmul(
                            accs[bk][:, sl * F1 : sl * F1 + bias_w],
                            lhsT=rdinvT_t[:, i * P : (i + 1) * P],
                            rhs=brow_t[:, bias_off : bias_off + bias_w],
                            start=(bank_seen[bk] == 0),
                            stop=(bank_seen[bk] == bank_tot[bk] - 1),
                        )
                        bank_seen[bk] += 1
                    for (c0, c1, pcs) in blk["chunks"]:
                        ncc = c1 - c0
                        stg = None
                        if "gather" not in skip or "matmul" not in skip:
                            stg = stgp.tile([P, ncc * F1], F16, tag="stg")
                        if "gather" not in skip:
                            for (s, pc0, pc1) in pcs:
                                nidx = (pc1 - pc0) * P
                                nc.gpsimd.dma_gather(
                                    stg[:, (pc0 - c0) * F1 : (pc1 - c0) * F1]
                                    .rearrange("p (c e) -> p c e", e=F1),
                                    table[WBASE[s] : WBASE[s] + WROWS[s], :],
                                    idx_t[:, pc0 * 8 : pc1 * 8],
                                    nidx, nidx_reg(nidx), F1,
                                    queue_num=piece_seq[0] % NQ,
                                )
                                piece_seq[0] += 1
                        S = None
                        if "select" not in skip or "matmul" not in skip:
                            S = selp.tile([P, ncc * P], F16, tag="S")
                        if "select" not in skip:
                            nc.vector.tensor_tensor(
                                out=S[:].rearrange("p (c q) -> p c q", q=P),
                                in0=dst_t[:, c0:c1].unsqueeze(2)
                                    .broadcast_to([P, ncc, P]),
                                in1=iota_t[:].unsqueeze(1)
                                    .broadcast_to([P, ncc, P]),
                                op=mybir.AluOpType.is_equal,
                            )
                        for c in range(c0, c1):
                            i = owner[c]
                            bk = bank_of[i]
                            sl = slice_of[i]
                            if "matmul" in skip:
                                bank_seen[bk] += 1
                                continue
                            nc.tensor.matmul(
                                accs[bk][:, sl * F1 : (sl + 1) * F1],
                                lhsT=S[:, (c - c0) * P : (c - c0 + 1) * P],
                                rhs=stg[:, (c - c0) * F1 : (c - c0 + 1) * F1],
                                start=(bank_seen[bk] == 0),
                                stop=(bank_seen[bk] == bank_tot[bk] - 1),
                            )
                            bank_seen[bk] += 1
                    for i in tiles:
                        bk, sl = bank_of[i], slice_of[i]
                        finish(i, accs[bk][:, sl * F1 : (sl + 1) * F1])
                    if bi in hooks:
                        hooks[bi]()

            def ln_chunk(s):
                """Batched bottleneck over window s's tiles: +bm, layernorm,
                then per tile q = dinv*(LN(z) @ W1d) -> bounce3; AG3 chunk."""
                T_ = WT[s]
                lo = WOFF[s]
                def r3(ap):
                    return ap.rearrange("p (t z) -> p t z", z=FZ)
                zmv = zm_all[:, lo * FZ : (lo + T_) * FZ]
                zbv = zb[:, lo * FZ : (lo + T_) * FZ]
                dfv = diff[:, lo * FZ : (lo + T_) * FZ]
                bm3 = bias_t[:, OBM : OBM + FZ].unsqueeze(1).broadcast_to([P, T_, FZ])
                nc.vector.tensor_tensor(
                    out=r3(zbv), in0=r3(zmv), in1=bm3, op=mybir.AluOpType.add)
                musum = workp.tile([P, T_], F32, tag="musum")
                nc.vector.reduce_sum(musum[:], r3(zbv), axis=mybir.AxisListType.X)
                mu = workp.tile([P, T_], F32, tag="mu")
                nc.vector.tensor_mul(out=mu[:], in0=musum[:],
                                     in1=inv5_t[:].broadcast_to([P, T_]))
                nc.vector.tensor_tensor(
                    out=r3(dfv), in0=r3(zbv),
                    in1=mu[:].unsqueeze(2).broadcast_to([P, T_, FZ]),
                    op=mybir.AluOpType.subtract)
                sq = workp.tile([P, T_ * FZ], F32, tag="sq")
                nc.vector.tensor_mul(out=sq[:], in0=dfv, in1=dfv)
                varsum = workp.tile([P, T_], F32, tag="varsum")
                nc.vector.reduce_sum(
                    varsum[:], sq[:].rearrange("p (t z) -> p t z", z=FZ),
                    axis=mybir.AxisListType.X)
                var = workp.tile([P, T_], F32, tag="var")
                nc.vector.tensor_mul(out=var[:], in0=varsum[:],
                                     in1=inv5_t[:].broadcast_to([P, T_]))
                vare = workp.tile([P, T_], F32, tag="vare")
                nc.vector.tensor_add(out=vare[:], in0=var[:],
                                     in1=eps_t[:].broadcast_to([P, T_]))
                sd = workp.tile([P, T_], F32, tag="sd")
                nc.scalar.activation(sd[:], vare[:], AF.Sqrt)
                rinv = workp.tile([P, T_], F32, tag="rinv")
                nc.vector.reciprocal(rinv[:], sd[:])
                zn = workp.tile([P, T_ * FZ], F32, tag="zn")
                nc.vector.tensor_tensor(
                    out=r3(zn[:]), in0=r3(dfv),
                    in1=rinv[:].unsqueeze(2).broadcast_to([P, T_, FZ]),
                    op=mybir.AluOpType.mult)
                zw = workp.tile([P, T_ * FZ], F32, tag="zw")
                nc.vector.tensor_tensor(
                    out=r3(zw[:]), in0=r3(zn[:]),
                    in1=bias_t[:, OLNW : OLNW + FZ].unsqueeze(1)
                        .broadcast_to([P, T_, FZ]),
                    op=mybir.AluOpType.mult)
                zl = workp.tile([P, T_ * FZ], F16, tag="zl")
                nc.vector.tensor_tensor(
                    out=r3(zl[:]), in0=r3(zw[:]),
                    in1=bias_t[:, OLNB : OLNB + FZ].unsqueeze(1)
                        .broadcast_to([P, T_, FZ]),
                    op=mybir.AluOpType.add)
                for k in range(T_):
                    i = lo + k
                    tr3 = psump.tile([FZ, P], F16, tag="tr")
                    nc.tensor.transpose(out=tr3[:], in_=zl[:, k * FZ:(k + 1) * FZ],
                                        identity=ident16[:])
                    zT = workp.tile([FZ, P], F16, tag="trs")
                    nc.vector.tensor_copy(out=zT[:], in_=tr3[:])
                    qp = psump.tile([P, F1], F32, tag="pmm")
                    nc.tensor.matmul(qp[:], lhsT=zT[:], rhs=w1d_t[:],
                                     start=True, stop=True)
                    q16 = workp.tile([P, F1], F16, tag="ms")
                    nc.scalar.activation(q16[:], qp[:], AF.Copy,
                                         scale=dinv_t[:, i : i + 1])
                    nc.sync.dma_start(out=bounce3[i * P : (i + 1) * P, :],
                                      in_=q16[:])
                ag_chunk(bounce3, t3, s)

            for _rep in range(reps):
                # ---- L1 produce: xT is resident, one matmul per tile
                for i in range(TILES_PER_CORE):
                    for s in range(3):
                        if i == WOFF[s + 1]:
                            ag_chunk(bounce1, t1, s)
                    mm = psump.tile([P, F1], F32, tag="pmm")
                    nc.tensor.matmul(
                        mm[:], lhsT=xT_t[:, i * P : (i + 1) * P], rhs=w1e_t[:],
                        start=True, stop=True)
                    ms = workp.tile([P, F1], F16, tag="ms")
                    nc.scalar.activation(ms[:], mm[:], AF.Copy,
                                         scale=dinv_t[:, i : i + 1])
                    nc.sync.dma_start(out=bounce1[i * P : (i + 1) * P, :],
                                      in_=ms[:])
                ag_chunk(bounce1, t1, 3)

                # ---- L1 aggregate -> h (relu) -> L2 produce (zero-padded)
                def fin1(i, acc):
                    h16 = epilogue(i, acc, F1, relu=True)
                    produce(i, h16, w2e_t, F2, bounce2)

                agg_layer(t1, fin1, {
                    HOOK_BLOCKS[0]: lambda: ag_chunk(bounce2, t2, 0),
                    HOOK_BLOCKS[1]: lambda: ag_chunk(bounce2, t2, 1),
                    HOOK_BLOCKS[2]: lambda: ag_chunk(bounce2, t2, 2),
                }, OB1, F1, skip)
                ag_chunk(bounce2, t2, 3)

                # ---- L2 aggregate -> z -> (relu z) @ Wm staged per tile
                def fin2(i, acc2):
                    zr16 = epilogue(i, acc2[:, :F2], F2, relu=True)
                    tr2 = psump.tile([F2, P], F16, tag="tr")
                    nc.tensor.transpose(out=tr2[:], in_=zr16[:],
                                        identity=ident16[:])
                    tr2s = workp.tile([F2, P], F16, tag="trs")
                    nc.vector.tensor_copy(out=tr2s[:], in_=tr2[:])
                    zm = psump.tile([P, FZ], F32, tag="pmm")
                    nc.tensor.matmul(zm[:], lhsT=tr2s[:], rhs=wm_t[:],
                                     start=True, stop=True)
                    nc.vector.tensor_copy(
                        out=zm_all[:, i * FZ : (i + 1) * FZ], in_=zm[:])

                agg_layer(t2, fin2, {
                    HOOK_BLOCKS[0]: lambda: ln_chunk(0),
                    HOOK_BLOCKS[1]: lambda: ln_chunk(1),
                    HOOK_BLOCKS[2]: lambda: ln_chunk(2),
                }, OB2, F2, skip)
                ln_chunk(3)

                # ---- L3 aggregate (q = dinv*(LN(z)@W1d) rows) -> d -> L4
                def fin3(i, acc3):
                    d16 = epilogue(i, acc3, F1, relu=True)
                    produce(i, d16, w2d_t, FO, bounce4)

                agg_layer(t3, fin3, {
                    HOOK_BLOCKS[0]: lambda: ag_chunk(bounce4, t4, 0),
                    HOOK_BLOCKS[1]: lambda: ag_chunk(bounce4, t4, 1),
                    HOOK_BLOCKS[2]: lambda: ag_chunk(bounce4, t4, 2),
                }, OB1D, F1, skip)
                ag_chunk(bounce4, t4, 3)

                # ---- L4 aggregate -> output
                def fin4(i, acc4):
                    o = epilogue(i, acc4, FO, relu=False)
                    nc.sync.dma_start(out=out_t[i * P : (i + 1) * P, :], in_=o[:])

                agg_layer(t4, fin4, {}, OB2D, FO, skip)
    return nc


# ------------------------------------------------------------------ kernel --

_CACHE = {}


def kernel(x, edge_index, W1e, b1e, W2e, b2e, Wm, bm, ln_w, ln_b,
           W1d, b1d, W2d, b2d):
    x = np.asarray(x, dtype=np.float32)
    edge_index = np.asarray(edge_index)
    plan = build_plan(edge_index)
    old_of_new = plan["old_of_new"]
    real = old_of_new >= 0

    # pack per-core inputs
    xg = np.zeros((NPAD, F1), np.float16)
    xg[real] = x[old_of_new[real]].astype(np.float16)
    bias_pack = np.zeros((P, F1 + F2 + FZ + F1 + FO + 2 * FZ), np.float32)
    o = 0
    for vec in (b1e, b2e, bm, b1d, b2d, ln_w, ln_b):
        v = np.asarray(vec, np.float32).ravel()
        bias_pack[:, o : o + v.size] = v[None, :]
        o += v.size
    iota = np.tile(np.arange(P, dtype=np.float16)[None, :], (P, 1))

    in_maps = []
    for c in range(NCORES):
        in_maps.append({
            "xT_shard": xg[c * SHARD : (c + 1) * SHARD].T.copy(),
            "idx": plan["idx_all"][c],
            "dstid": plan["dst_f16"][c],
            "iota_in": iota,
            "dinv_cols": plan["dinv_cols"][c],
            "rdinv_flat": plan["rdinv_flat"][c],
            "biasrow": bias_pack[:1].astype(np.float16),
            "W1e": np.asarray(W1e, np.float16),
            "W2e": np.asarray(W2e, np.float16),
            "Wm": np.asarray(Wm, np.float16),
            "W1d": np.asarray(W1d, np.float16),
            "W2d": np.asarray(W2d, np.float16),
            "biases": bias_pack,
        })

    key = plan["colspec"]
    if key not in _CACHE:
        nc = build_program(plan["colspec"])
        _CACHE[key] = SpmdRunner(nc)
    runner = _CACHE[key]
    runner.stage(in_maps)
    res = runner.results(runner.run())

    out_new = np.concatenate([res[c]["out"] for c in range(NCORES)], axis=0)
    out = np.zeros((N, FO), np.float32)
    out[old_of_new[real]] = out_new[real]
    return out
